# revision 12
# baseline (speedup 1.0000x reference)
"""EngramANNInjection Trainium2 kernel (8-core SPMD, data-parallel over tokens).

Each core handles 512 tokens (one half-sequence: core c -> batch c//2,
half c%2) against the full 32768-entry table, flash-softmax style.
The 2-token causal-conv halo between half-sequences is exchanged with a
pair-wise AllGather ([0,1],[2,3],...).

HW-validated constraints honored here:
  - tensor_tensor_reduce crashes the runtime -> use ACT Square accum_out
    or mult+reduce_sum instead.
  - f32r matmuls need an even moving free dim (table rhs padded to 258).
  - DVE can read only one PSUM operand per instruction.
  - CoreSim lacks Silu -> Sigmoid + mul.
  - activation float bias needs a registered const AP -> pass an AP.
"""
import sys
import os

for _p in ("/opt/trn_rl_repo",):
    if _p not in sys.path and os.path.isdir(_p):
        sys.path.append(_p)

import numpy as np
import concourse.bass as bass
import concourse.mybir as mybir
import concourse.tile as tile
from concourse import bacc
from concourse.bass_utils import run_bass_kernel_spmd
from concourse.masks import make_identity

F32 = mybir.dt.float32
F32R = mybir.dt.float32r
ALU = mybir.AluOpType
ACTF = mybir.ActivationFunctionType

P = 128
E = 256          # engram dim
EC = 2           # e chunks
C = 2048         # hidden dim
CC = 16          # c chunks
T = 32768        # table entries
NT = 256         # entry tiles
TOK = 512        # tokens per core
MC = 4           # token chunks
K = 3            # conv taps
LN_EPS = 1e-6
NORM_EPS = 1e-8
INV_TEMP = float(E) ** 0.5          # 16.0
INV_SQRT_C = 1.0 / float(np.sqrt(C))
CLAMP_MIN = 0.5
USE_AG = os.environ.get("K_AG", "1") == "1"
USE_GBCAST = os.environ.get("K_GB", "1") == "1"


def build():
    nc = bacc.Bacc(None, num_devices=8, target_bir_lowering=False)

    # ---- per-core inputs (partition-first layouts, host-prepped) ----
    hT = nc.dram_tensor("hT", [P, CC, TOK], F32, kind="ExternalInput")       # [p, kc, t] = hid[t, kc*128+p]
    h = nc.dram_tensor("h", [P, MC, C], F32, kind="ExternalInput")           # [p, mc, c] = hid[mc*128+p, c]
    tTr = nc.dram_tensor("tTr", [P, NT, EC, P], F32, kind="ExternalInput")   # [p, ti, ec, tq] = tbl[ti*128+tq, ec*128+p]
    tb1 = nc.dram_tensor("tb1", [P, NT, E + 2], F32, kind="ExternalInput")   # [p, ti, :] = [tbl[ti*128+p, :], 1, 0]
    wq = nc.dram_tensor("wq", [P, CC, E], F32, kind="ExternalInput")         # [p, kc, e] = Wq[kc*128+p, e]
    wk = nc.dram_tensor("wk", [P, EC, C], F32, kind="ExternalInput")         # [p, ec, c] = Wk[ec*128+p, c]
    wv = nc.dram_tensor("wv", [P, EC, C], F32, kind="ExternalInput")         # [p, ec, c] = Wv[ec*128+p, c]
    gam = nc.dram_tensor("gam", [P, E], F32, kind="ExternalInput")           # ln_gamma replicated
    bet = nc.dram_tensor("bet", [P, E], F32, kind="ExternalInput")           # ln_beta replicated
    gwr = nc.dram_tensor("gwr", [P, C], F32, kind="ExternalInput")           # gate_norm_w replicated
    kwr = nc.dram_tensor("kwr", [P, C], F32, kind="ExternalInput")           # key_norm_w replicated
    cw = nc.dram_tensor("cw", [P, CC, K], F32, kind="ExternalInput")         # [p, cc, k] = conv_w[cc*128+p, k]
    msk = nc.dram_tensor("msk", [P, 1], F32, kind="ExternalInput")           # 1.0 if odd half else 0.0

    outT = nc.dram_tensor("outT", [P, CC, TOK], F32, kind="ExternalOutput")  # [p, cc, t] = out[t, cc*128+p]
    gout = nc.dram_tensor("gout", [P, MC], F32, kind="ExternalOutput")       # [p, mc] = gate[mc*128+p]

    with tile.TileContext(nc) as tc:
        with (
            tc.tile_pool(name="const", bufs=1) as cst,
            tc.tile_pool(name="dram", bufs=1, space="DRAM") as dram,
        ):
            ident = cst.tile([P, P], F32)
            make_identity(nc, ident[:])
            gam_sb = cst.tile([P, E], F32)
            bet_sb = cst.tile([P, E], F32)
            u_sb = cst.tile([P, C], F32)
            kw_sb = cst.tile([P, C], F32)
            cw_sb = cst.tile([P, CC, K], F32)
            msk_sb = cst.tile([P, 1], F32)
            eps_sb = cst.tile([P, 1], F32)
            nc.any.memset(eps_sb[:], LN_EPS)
            nc.sync.dma_start(gam_sb[:], gam.ap())
            nc.sync.dma_start(bet_sb[:], bet.ap())
            nc.sync.dma_start(u_sb[:], gwr.ap())
            nc.sync.dma_start(kw_sb[:], kwr.ap())
            nc.sync.dma_start(cw_sb[:], cw.ap())
            nc.sync.dma_start(msk_sb[:], msk.ap())
            # u = gate_norm_w * key_norm_w (elementwise, replicated)
            nc.vector.tensor_tensor(u_sb[:], u_sb[:], kw_sb[:], ALU.mult)

            qT = [cst.tile([P, TOK], F32R, tag=f"qT{ec}", name=f"qT{ec}") for ec in range(EC)]
            rT = [cst.tile([P, TOK], F32R, tag=f"rT{ec}", name=f"rT{ec}") for ec in range(EC)]
            wk_sb = cst.tile([P, EC, C], F32R)
            wv_sb = cst.tile([P, EC, C], F32R)
            nc.sync.dma_start(wk_sb[:], wk.ap().bitcast(F32R))
            nc.sync.dma_start(wv_sb[:], wv.ap().bitcast(F32R))
            gate_sb = cst.tile([P, MC], F32)
            gate_rep = cst.tile([P, TOK], F32)
            # gT_ext[p, cc, 2 + t] = gated^T, cols 0:2 = halo
            gT_ext = cst.tile([P, CC, 2 + TOK], F32)

            # ======= phase Q: q = hid @ Wq, l2norm, transpose =======
            with (
                tc.tile_pool(name="qpool", bufs=3) as qp,
                tc.tile_pool(name="qpsum", bufs=1, space="PSUM") as qps,
                tc.tile_pool(name="tpsum", bufs=2, space="PSUM") as tps,
            ):
                psQ = [qps.tile([P, E], F32, tag=f"psQ{mc}", name=f"psQ{mc}") for mc in range(MC)]
                for kc in range(CC):
                    hT_t = qp.tile([P, TOK], F32R, tag="hT")
                    wq_t = qp.tile([P, E], F32R, tag="wq")
                    nc.sync.dma_start(hT_t[:], hT.ap()[:, kc].bitcast(F32R))
                    nc.sync.dma_start(wq_t[:], wq.ap()[:, kc].bitcast(F32R))
                    for mc in range(MC):
                        nc.tensor.matmul(
                            psQ[mc][:],
                            hT_t[:, bass.ts(mc, P)],
                            wq_t[:],
                            start=(kc == 0),
                            stop=(kc == CC - 1),
                        )
                for mc in range(MC):
                    scr = qp.tile([P, E], F32, tag="scr")
                    qss = qp.tile([P, 1], F32, tag="qss")
                    nc.scalar.activation(scr[:], psQ[mc][:], ACTF.Square, accum_out=qss[:])
                    qn = qp.tile([P, 1], F32, tag="qn")
                    nc.scalar.activation(qn[:], qss[:], ACTF.Sqrt)
                    nc.vector.tensor_scalar_max(qn[:], qn[:], NORM_EPS)
                    invq = qp.tile([P, 1], F32, tag="invq")
                    nc.vector.reciprocal(invq[:], qn[:])
                    qhat = qp.tile([P, E], F32, tag="qhat")
                    nc.vector.tensor_scalar_mul(qhat[:], psQ[mc][:], invq[:])
                    for ec in range(EC):
                        psT = tps.tile([P, P], F32, tag="psT")
                        nc.tensor.transpose(psT[:], qhat[:, bass.ts(ec, P)], ident[:])
                        nc.any.tensor_copy(qT[ec][:, bass.ts(mc, P)], psT[:])

            # ======= flash loop over entry tiles =======
            with (
                tc.tile_pool(name="fpool", bufs=4) as fp,
                tc.tile_pool(name="ppool", bufs=3) as pp,
                tc.tile_pool(name="apsum", bufs=1, space="PSUM") as aps,
                tc.tile_pool(name="spsum", bufs=2, space="PSUM") as sps,
            ):
                psA = [aps.tile([P, E + 2], F32, tag=f"psA{mc}", name=f"psA{mc}") for mc in range(MC)]
                for ti in range(NT):
                    tT_t = fp.tile([P, EC, P], F32R, tag="tT")
                    tb_t = fp.tile([P, E + 2], F32R, tag="tb")
                    nc.sync.dma_start(tT_t[:], tTr.ap()[:, ti].bitcast(F32R))
                    nc.sync.dma_start(tb_t[:], tb1.ap()[:, ti].bitcast(F32R))
                    # 16/|t| per entry (exp scale; folds table l2norm + temperature)
                    scr = pp.tile([P, E], F32, tag="fscr")
                    tss = pp.tile([P, 1], F32, tag="tss")
                    nc.scalar.activation(scr[:], tb_t[:, :E].bitcast(F32), ACTF.Square, accum_out=tss[:])
                    scl = pp.tile([P, 1], F32, tag="fscl")
                    nc.scalar.activation(scl[:], tss[:], ACTF.Sqrt, scale=1.0 / (INV_TEMP * INV_TEMP))
                    nc.vector.tensor_scalar_max(scl[:], scl[:], NORM_EPS / INV_TEMP)
                    nc.vector.reciprocal(scl[:], scl[:])
                    # S^T = table_tile . qhat  -> [ent, tok]
                    psS = sps.tile([P, TOK], F32, tag="psS")
                    for ec in range(EC):
                        nc.tensor.matmul(
                            psS[:], tT_t[:, ec], qT[ec][:],
                            start=(ec == 0), stop=(ec == EC - 1),
                        )
                    pexp = pp.tile([P, TOK], F32R, tag="pexp")
                    nc.scalar.activation(pexp[:], psS[:], ACTF.Exp, scale=scl[:])
                    for mc in range(MC):
                        nc.tensor.matmul(
                            psA[mc][:],
                            pexp[:, bass.ts(mc, P)],
                            tb_t[:],
                            start=(ti == 0),
                            stop=(ti == NT - 1),
                        )

                # ======= post: retrieved, layernorm, transpose =======
                with (
                    tc.tile_pool(name="opool", bufs=2) as op,
                    tc.tile_pool(name="tpsum2", bufs=2, space="PSUM") as tps2,
                ):
                    for mc in range(MC):
                        invd = op.tile([P, 1], F32, tag="invd")
                        nc.vector.reciprocal(invd[:], psA[mc][:, E:E + 1])
                        r_sb = op.tile([P, E], F32, tag="r_sb")
                        nc.vector.tensor_scalar_mul(r_sb[:], psA[mc][:, :E], invd[:])
                        mu = op.tile([P, 1], F32, tag="mu")
                        nc.vector.tensor_reduce(mu[:], r_sb[:], axis=mybir.AxisListType.X, op=ALU.add)
                        nc.vector.tensor_scalar_mul(mu[:], mu[:], 1.0 / E)
                        xm = op.tile([P, E], F32, tag="xm")
                        nc.vector.tensor_scalar_sub(xm[:], r_sb[:], mu[:])
                        vscr = op.tile([P, E], F32, tag="vscr")
                        vac = op.tile([P, 1], F32, tag="vac")
                        nc.scalar.activation(vscr[:], xm[:], ACTF.Square, accum_out=vac[:])
                        sd = op.tile([P, 1], F32, tag="sd")
                        nc.scalar.activation(sd[:], vac[:], ACTF.Sqrt, scale=1.0 / E, bias=eps_sb[:])
                        rstd = op.tile([P, 1], F32, tag="rstd")
                        nc.vector.reciprocal(rstd[:], sd[:])
                        rln = op.tile([P, E], F32, tag="rln")
                        nc.vector.tensor_scalar_mul(rln[:], xm[:], rstd[:])
                        nc.vector.tensor_tensor(rln[:], rln[:], gam_sb[:], ALU.mult)
                        nc.vector.tensor_tensor(rln[:], rln[:], bet_sb[:], ALU.add)
                        for ec in range(EC):
                            psT2 = tps2.tile([P, P], F32, tag="psT2")
                            nc.tensor.transpose(psT2[:], rln[:, bass.ts(ec, P)], ident[:])
                            nc.any.tensor_copy(rT[ec][:, bass.ts(mc, P)], psT2[:])

            # ======= key projection + gate =======
            with (
                tc.tile_pool(name="kpool", bufs=2) as kp,
                tc.tile_pool(name="kpsum", bufs=2, space="PSUM") as kps,
            ):
                NC4 = C // TOK  # 4 chunks of 512 along C
                for mc in range(MC):
                    h_sb = kp.tile([P, C], F32, tag="h_sb")
                    nc.sync.dma_start(h_sb[:], h.ap()[:, mc])
                    hu = kp.tile([P, C], F32, tag="hu")
                    nc.vector.tensor_tensor(hu[:], h_sb[:], u_sb[:], ALU.mult)
                    sscr = kp.tile([P, C], F32, tag="sscr")
                    ssh = kp.tile([P, 1], F32, tag="ssh")
                    nc.scalar.activation(sscr[:], h_sb[:], ACTF.Square, accum_out=ssh[:])
                    dotp = [kp.tile([P, 1], F32, tag=f"dotp{j}", name=f"dotp{j}") for j in range(NC4)]
                    sskp = [kp.tile([P, 1], F32, tag=f"sskp{j}", name=f"sskp{j}") for j in range(NC4)]
                    for j in range(NC4):
                        psK = kps.tile([P, TOK], F32, tag="psK")
                        for ec in range(EC):
                            nc.tensor.matmul(
                                psK[:],
                                rT[ec][:, bass.ts(mc, P)],
                                wk_sb[:, ec, bass.ts(j, TOK)],
                                start=(ec == 0), stop=(ec == EC - 1),
                            )
                        kscr = kp.tile([P, TOK], F32, tag="kscr")
                        nc.vector.tensor_tensor(kscr[:], psK[:], hu[:, bass.ts(j, TOK)], ALU.mult)
                        nc.vector.tensor_reduce(dotp[j][:], kscr[:], axis=mybir.AxisListType.X, op=ALU.add)
                        kscr2 = kp.tile([P, TOK], F32, tag="kscr2")
                        nc.scalar.activation(kscr2[:], psK[:], ACTF.Square, accum_out=sskp[j][:])
                    for j in range(1, NC4):
                        nc.vector.tensor_tensor(sskp[j][:], sskp[j][:], sskp[j - 1][:], ALU.add)
                        nc.vector.tensor_tensor(dotp[j][:], dotp[j][:], dotp[j - 1][:], ALU.add)
                    # gate = max(sigmoid(dot_norm / sqrt(C)), 0.5)
                    a_t = kp.tile([P, 1], F32, tag="a_t")
                    b_t = kp.tile([P, 1], F32, tag="b_t")
                    nc.vector.tensor_scalar(a_t[:], ssh[:], 1.0 / C, LN_EPS, ALU.mult, ALU.add)
                    nc.vector.tensor_scalar(b_t[:], sskp[NC4 - 1][:], 1.0 / C, LN_EPS, ALU.mult, ALU.add)
                    nc.vector.tensor_tensor(a_t[:], a_t[:], b_t[:], ALU.mult)
                    sp_t = kp.tile([P, 1], F32, tag="sp_t")
                    nc.scalar.activation(sp_t[:], a_t[:], ACTF.Sqrt)
                    rp_t = kp.tile([P, 1], F32, tag="rp_t")
                    nc.vector.reciprocal(rp_t[:], sp_t[:])
                    dn_t = kp.tile([P, 1], F32, tag="dn_t")
                    nc.vector.tensor_tensor(dn_t[:], dotp[NC4 - 1][:], rp_t[:], ALU.mult)
                    nc.scalar.activation(gate_sb[:, mc:mc + 1], dn_t[:], ACTF.Sigmoid, scale=INV_SQRT_C)
                    nc.vector.tensor_scalar_max(gate_sb[:, mc:mc + 1], gate_sb[:, mc:mc + 1], CLAMP_MIN)
                nc.sync.dma_start(gout.ap(), gate_sb[:])
                if USE_GBCAST:
                    # broadcast gate over partitions: SBUF->DRAM->SBUF, 0-step src
                    g_dram = dram.tile([MC, P], F32)
                    nc.sync.dma_start(g_dram[:].rearrange("m q -> q m"), gate_sb[:])
                    nc.sync.dma_start(
                        gate_rep[:],
                        g_dram[:].rearrange("m q -> (m q)").partition_broadcast(P),
                    )
                else:
                    nc.any.memset(gate_rep[:], 1.0)

            # ======= value^T, gating, AllGather halo, conv, sigmoid*x =======
            with (
                tc.tile_pool(name="vpool", bufs=3) as vp,
                tc.tile_pool(name="vpsum", bufs=2, space="PSUM") as vps,
            ):
                for cc in range(CC):
                    psV = vps.tile([P, TOK], F32, tag="psV")
                    for ec in range(EC):
                        nc.tensor.matmul(
                            psV[:],
                            wv_sb[:, ec, bass.ts(cc, P)],
                            rT[ec][:],
                            start=(ec == 0), stop=(ec == EC - 1),
                        )
                    nc.vector.tensor_tensor(gT_ext[:, cc, 2:], psV[:], gate_rep[:], ALU.mult)

                if USE_AG:
                    cin = dram.tile([P, CC, 2], F32)
                    cout = dram.tile([2, P, CC, 2], F32)
                    nc.gpsimd.dma_start(cin[:], gT_ext[:, :, TOK:TOK + 2])
                    nc.gpsimd.collective_compute(
                        "AllGather",
                        ALU.bypass,
                        replica_groups=[[0, 1], [2, 3], [4, 5], [6, 7]],
                        ins=[cin.opt()],
                        outs=[cout.opt()],
                    )
                    halo = vp.tile([P, CC, 2], F32)
                    nc.gpsimd.dma_start(halo[:], cout[0])
                    nc.vector.tensor_scalar_mul(gT_ext[:, :, 0:2], halo[:], msk_sb[:])
                else:
                    nc.any.memset(gT_ext[:, :, 0:2], 0.0)

                for cc in range(CC):
                    cv = vp.tile([P, TOK], F32, tag="cv")
                    nc.vector.tensor_scalar_mul(cv[:], gT_ext[:, cc, 0:TOK], cw_sb[:, cc, 0:1])
                    for k in range(1, K):
                        tmp = vp.tile([P, TOK], F32, tag="cvt")
                        nc.vector.tensor_scalar_mul(tmp[:], gT_ext[:, cc, k:k + TOK], cw_sb[:, cc, k:k + 1])
                        nc.vector.tensor_tensor(cv[:], cv[:], tmp[:], ALU.add)
                    osb = vp.tile([P, TOK], F32, tag="osb")
                    nc.scalar.activation(osb[:], cv[:], ACTF.Sigmoid)
                    nc.vector.tensor_tensor(osb[:], osb[:], cv[:], ALU.mult)
                    nc.sync.dma_start(outT.ap()[:, cc], osb[:])

    nc.finalize()
    return nc


_NC_CACHE = None


def _get_nc():
    global _NC_CACHE
    if _NC_CACHE is None:
        _NC_CACHE = build()
    return _NC_CACHE


def _prep_core_inputs(hidden_state, table, Wq, ln_gamma, ln_beta, Wk, Wv,
                      gate_norm_w, key_norm_w, conv_w):
    """Build the 8 per-core input dicts (layout/shard only, no math
    beyond concatenating the ones/zeros padding columns)."""
    f = np.float32
    tbl = np.ascontiguousarray(table, dtype=f)
    tTr = np.ascontiguousarray(
        tbl.reshape(NT, P, EC, P).transpose(3, 0, 2, 1))
    tb1 = np.concatenate([tbl, np.ones((T, 1), f), np.zeros((T, 1), f)], axis=1)
    tb1 = np.ascontiguousarray(tb1.reshape(NT, P, E + 2).transpose(1, 0, 2))
    wq_r = np.ascontiguousarray(np.asarray(Wq, f).reshape(CC, P, E).transpose(1, 0, 2))
    wk_r = np.ascontiguousarray(np.asarray(Wk, f).reshape(EC, P, C).transpose(1, 0, 2))
    wv_r = np.ascontiguousarray(np.asarray(Wv, f).reshape(EC, P, C).transpose(1, 0, 2))
    gam = np.ascontiguousarray(np.broadcast_to(np.asarray(ln_gamma, f), (P, E)))
    bet = np.ascontiguousarray(np.broadcast_to(np.asarray(ln_beta, f), (P, E)))
    gwr = np.ascontiguousarray(np.broadcast_to(np.asarray(gate_norm_w, f), (P, C)))
    kwr = np.ascontiguousarray(np.broadcast_to(np.asarray(key_norm_w, f), (P, C)))
    cw_r = np.ascontiguousarray(np.asarray(conv_w, f).reshape(CC, P, K).transpose(1, 0, 2))
    hs = np.asarray(hidden_state, f)
    in_maps = []
    for core in range(8):
        b, half = core // 2, core % 2
        hid = hs[b, half * TOK:(half + 1) * TOK]          # [512, 2048]
        hT_c = np.ascontiguousarray(hid.T.reshape(CC, P, TOK).transpose(1, 0, 2))
        h_c = np.ascontiguousarray(hid.reshape(MC, P, C).transpose(1, 0, 2))
        in_maps.append({
            "hT": hT_c, "h": h_c, "tTr": tTr, "tb1": tb1,
            "wq": wq_r, "wk": wk_r, "wv": wv_r,
            "gam": gam, "bet": bet, "gwr": gwr, "kwr": kwr, "cw": cw_r,
            "msk": np.full((P, 1), float(half), f),
        })
    return in_maps


def _assemble(results):
    out = np.empty((4, 1024, C), np.float32)
    gate = np.empty((4, 1024, 1), np.float32)
    for core in range(8):
        b, half = core // 2, core % 2
        oT = results[core]["outT"]                        # [p, cc, t]
        out[b, half * TOK:(half + 1) * TOK] = (
            oT.transpose(2, 1, 0).reshape(TOK, C))
        g = results[core]["gout"]                         # [p, mc]
        gate[b, half * TOK:(half + 1) * TOK, 0] = g.T.reshape(TOK)
    return out, gate


def run_on_hw(in_maps, trace=False):
    nc = _get_nc()
    if trace:
        _install_profile_shim()
    return run_bass_kernel_spmd(nc, in_maps, core_ids=list(range(8)), trace=trace)


def _install_profile_shim():
    """antenv.axon_hooks is absent in this image; recreate it so
    run_bass_kernel_spmd(trace=True) can reach the NTFF profiler."""
    import types
    try:
        import antenv
        if "antenv.axon_hooks" not in sys.modules:
            m = types.ModuleType("antenv.axon_hooks")
            m._hook = None
            m.set_axon_ntff_profile_hook = lambda hk: setattr(m, "_hook", hk)
            m.get_axon_ntff_profile_hook = lambda: m._hook
            sys.modules["antenv.axon_hooks"] = m
            antenv.axon_hooks = m
        from trn_agent_boot.trn_boot import _ntff_profile_via_ctypes
        from antenv.axon_hooks import set_axon_ntff_profile_hook
        set_axon_ntff_profile_hook(
            _ntff_profile_via_ctypes("/opt/axon/libaxon_pjrt.so"))
    except Exception:
        pass


def kernel(**inputs):
    in_maps = _prep_core_inputs(**inputs)
    r = run_on_hw(in_maps, trace=False)
    return _assemble(r.results)
